# revision 6
# baseline (speedup 1.0000x reference)
"""GAT conv layer (B=2, N=4096, C=256, H=4, D=64) on TRN2 NeuronCores.

Execution-environment reality (measured): each STATIC instruction in the
NEFF costs ~20-30us per execution of the (replicated) pipeline, regardless
of engine; instructions inside For_i hardware loops pay that once plus an
all-engine barrier (~3.4us) per loop ITERATION, with near-architectural
marginal per-element rates.  Static instruction count is therefore the
cost function.  This version fuses projection + score/attention into ONE
64-iteration hardware loop (2 head-pair passes x 32 source chunks, pass =
t>>5, jc = t&31), unrolls the tiny srcB phase (4x8 instrs < loop
machinery), and normalizes in [d, i] layout (host does the final layout
transpose).  Sharding: 8 cores = (batch b, target-quarter iq).

PSUM budget per loop iteration: 2 head accumulators [66, 1024] (4 banks)
+ projection psum [128, 132] (1 bank).  The pass boundary needs no If:
the accumulators are drained to SBUF every iteration (last write of each
pass wins) and pass 1 accumulates on top of pass 0; head {2,3} results
are recovered by two subtractions afterwards.

Per-iteration body (t -> pass, jc): stage x-chunk (2 DVE copies), project
[0.25*W|w_tgt|0] for the pass's 2 heads (2 matmuls, f16), stage to wc +
set ones columns; DMA mask chunk; per head hh in {0,1}:
z = (srcB + tgt[j]) - 255*mask (STT), l = max(.2z,z) (STT), p = Exp(l)
(ACT), acc_hh += [W|tgt|1].T @ p (2 matmuls, rows 0-63 = numerator,
64 = softmax denominator); drain acc -> nd[pass].
"""

import numpy as np

B, N, C, H, D = 2, 4096, 256, 4, 64
NEG = 0.2
JC = N // 128        # 32 source chunks
NQ = N // 4          # 1024 target nodes per core
NCORES = 8

_cached = {}


def _build(reps=1):
    import concourse.bacc as bacc
    import concourse.tile as tile
    from concourse import mybir
    from concourse.bass import ds
    from concourse.masks import make_identity

    f32 = mybir.dt.float32
    f16 = mybir.dt.float16
    u8 = mybir.dt.uint8
    Alu = mybir.AluOpType
    Exp = mybir.ActivationFunctionType.Exp

    nc = bacc.Bacc(None, target_bir_lowering=False, name="gat4")

    xTd = nc.dram_tensor("xT", [128, 2 * N], f16, kind="ExternalInput")
    xTid = nc.dram_tensor("xTi", [128, 2 * NQ], f16, kind="ExternalInput")
    # waug: [pass(2), cc(2), hh(2)*66] = [0.25*W_h | w_tgt_h | 0], h=2*pass+hh
    waugd = nc.dram_tensor("waug", [128, 2 * 264], f16, kind="ExternalInput")
    wsbd = nc.dram_tensor("wsb", [128, H * 256], f16, kind="ExternalInput")
    mprepd = nc.dram_tensor("mprep", [128, JC * NQ], u8, kind="ExternalInput")
    outd = nc.dram_tensor("out", [64, NQ], f32, kind="ExternalOutput")

    def pipeline(tc, ones64):
        with tc.tile_pool(name="seq", bufs=1) as seq:
            xT = seq.tile([128, 2 * N], f16)
            xTi = seq.tile([128, 2 * NQ], f16)
            waug = seq.tile([128, 528], f16)
            wsb = seq.tile([128, H * 256], f16)
            srcB = seq.tile([128, H * NQ], f16)
            nd = seq.tile([66, 4 * NQ], f32)      # [pass, hh, NQ]
            nc.sync.dma_start(xT, xTd[:, :])
            nc.sync.dma_start(xTi, xTid[:, :])
            nc.sync.dma_start(waug, waugd[:, :])
            nc.sync.dma_start(wsb, wsbd[:, :])

            # ------------- phase B (unrolled): srcB broadcast -------------
            with tc.tile_pool(name="psB", bufs=1, space="PSUM") as psB:
                pss = [psB.tile([128, 512], f32, name=f"pss{q}")
                       for q in range(2)]
                for h in range(H):
                    for q in range(2):
                        nc.tensor.matmul(
                            pss[q], wsb[:, h * 256:h * 256 + 128],
                            xTi[:, q * 512:(q + 1) * 512],
                            start=True, stop=False)
                        nc.tensor.matmul(
                            pss[q], wsb[:, h * 256 + 128:(h + 1) * 256],
                            xTi[:, NQ + q * 512:NQ + (q + 1) * 512],
                            start=False, stop=True)
                        nc.scalar.copy(srcB[:, h * NQ + q * 512:
                                            h * NQ + (q + 1) * 512], pss[q])

            # ------- fused A+C: one For_i(32) loop per head-pair pass -----
            with tc.tile_pool(name="mw", bufs=1) as mw, \
                 tc.tile_pool(name="pc", bufs=1) as pc, \
                 tc.tile_pool(name="psC", bufs=1, space="PSUM") as psC:
                acc = [psC.tile([66, NQ], f32, name=f"acc{hh}")
                       for hh in range(2)]
                psp = psC.tile([128, 132], f32, name="psp")
                xcur = pc.tile([128, 256], f16)
                wc = pc.tile([128, 132], f16)
                m_t = mw.tile([128, NQ], u8)
                zts = [pc.tile([128, NQ], f16, name=f"z{hh}") for hh in range(2)]
                lts = [pc.tile([128, NQ], f16, name=f"l{hh}") for hh in range(2)]
                pts = [pc.tile([128, NQ], f16, name=f"p{hh}") for hh in range(2)]
                for ps in range(2):
                    for hh in range(2):
                        nc.vector.memset(acc[hh], 0.0)
                    with tc.For_i(0, JC) as jc:
                        nc.sync.dma_start(m_t, mprepd[:, ds(jc * NQ, NQ)])
                        nc.vector.tensor_copy(xcur[:, 0:128],
                                              xT[:, ds(jc * 128, 128)])
                        nc.vector.tensor_copy(xcur[:, 128:256],
                                              xT[:, ds(N + jc * 128, 128)])
                        nc.tensor.matmul(psp, xcur[:, 0:128],
                                         waug[:, ps * 264:ps * 264 + 132],
                                         start=True, stop=False)
                        nc.tensor.matmul(psp, xcur[:, 128:256],
                                         waug[:, ps * 264 + 132:ps * 264 + 264],
                                         start=False, stop=True)
                        nc.scalar.copy(wc, psp)
                        nc.vector.memset(
                            wc.rearrange("p (g l) -> p g l", l=66)[:, :, 64:65],
                            1.0)
                        for hh in range(2):
                            h = 2 * ps + hh
                            nc.vector.scalar_tensor_tensor(
                                out=zts[hh],
                                in0=srcB[:, h * NQ:(h + 1) * NQ],
                                scalar=wc[:, hh * 66 + 65:hh * 66 + 66],
                                in1=m_t, op0=Alu.add, op1=Alu.subtract)
                            nc.vector.scalar_tensor_tensor(
                                out=lts[hh], in0=zts[hh], scalar=NEG,
                                in1=zts[hh], op0=Alu.mult, op1=Alu.max)
                            nc.scalar.activation(out=pts[hh], in_=lts[hh],
                                                 func=Exp)
                            for q in range(2):
                                nc.tensor.matmul(
                                    acc[hh][:, q * 512:(q + 1) * 512],
                                    wc[:, hh * 66:(hh + 1) * 66],
                                    pts[hh][:, q * 512:(q + 1) * 512],
                                    start=False, stop=False,
                                    skip_group_check=True)
                    for hh in range(2):
                        h = 2 * ps + hh
                        nc.scalar.copy(nd[:, h * NQ:(h + 1) * NQ], acc[hh])

            # ------------- phase D: normalize in [d, i] layout ------------
            with tc.tile_pool(name="pd", bufs=1) as pd, \
                 tc.tile_pool(name="psD", bufs=1, space="PSUM") as psD:
                heads = [nd[:, h * NQ:(h + 1) * NQ] for h in range(H)]
                rr = pd.tile([1, H * NQ], f32)
                obT = pd.tile([64, NQ], f32)
                rb = [psD.tile([64, 512], f32, name=f"rb{h}")
                      for h in range(H)]
                tmp = [pd.tile([64, 512], f32, name=f"tmp{h}")
                       for h in range(3)]
                for q in range(2):
                    sl = slice(q * 512, (q + 1) * 512)
                    for h in range(H):
                        with nc.allow_low_precision(reason="softmax denom"):
                            nc.vector.reciprocal(
                                rr[:, h * NQ + q * 512:h * NQ + (q + 1) * 512],
                                heads[h][64:65, sl])
                        nc.tensor.matmul(
                            rb[h], ones64,
                            rr[:, h * NQ + q * 512:h * NQ + (q + 1) * 512],
                            start=True, stop=True, skip_group_check=True)
                    for h in range(H):
                        dst = tmp[h - 1] if h else obT[:, sl]
                        nc.vector.tensor_mul(dst, heads[h][0:64, sl], rb[h])
                        if h:
                            src0 = obT[:, sl] if h == 1 else tmp[h - 2]
                            dst2 = obT[:, sl] if h == 3 else tmp[h - 1]
                            nc.vector.tensor_add(dst2, src0, tmp[h - 1])
                nc.sync.dma_start(outd[:, :], obT)

    with tile.TileContext(nc) as tc:
        with tc.tile_pool(name="const", bufs=1) as const:
            ones64 = const.tile([1, 64], f32)
            nc.vector.memset(ones64, 1.0)
            for _rep in range(reps):
                pipeline(tc, ones64)

    nc.compile()
    return nc


def _prep_inputs(x, adj_matrix_masked, W, attention):
    """Host-side shard/layout prep (slicing, transposes, weight packing)."""
    x = np.ascontiguousarray(x, dtype=np.float32)
    W = np.ascontiguousarray(W, dtype=np.float32)
    attention = np.ascontiguousarray(attention, dtype=np.float32)

    a_src = attention[:, :D, 0]          # [H, D]
    a_tgt = attention[:, D:, 0]          # [H, D]
    Wh_cols = W.reshape(C, H, D)
    w_src = np.einsum("chd,hd->ch", Wh_cols, a_src)   # [C, H]
    w_tgt = np.einsum("chd,hd->ch", Wh_cols, a_tgt)   # [C, H]

    # waug[p, pass*264 + cc*132 + hh*66 + col], h = 2*pass + hh
    waug16 = np.zeros((128, 528), np.float16)
    for p in range(2):
        for cc in range(2):
            for hh in range(2):
                h = 2 * p + hh
                base = p * 264 + cc * 132 + hh * 66
                rows = slice(cc * 128, (cc + 1) * 128)
                waug16[:, base:base + 64] = 0.25 * Wh_cols[rows, h, :]
                waug16[:, base + 65] = w_tgt[rows, h]
    # wsb: [128, h*256 + cc*128 + k] = w_src[cc*128+p, h]  (repeated over k)
    wsb16 = np.empty((128, H * 256), np.float16)
    for h in range(H):
        for cc in range(2):
            wsb16[:, h * 256 + cc * 128: h * 256 + (cc + 1) * 128] = \
                w_src[cc * 128:(cc + 1) * 128, h][:, None]

    in_maps = []
    for b in range(B):
        # xT16[p, cc*N + j] = x[b, j, cc*128+p]
        xT16 = np.ascontiguousarray(
            x[b].T.reshape(2, 128, N).transpose(1, 0, 2).reshape(128, 2 * N)
        ).astype(np.float16)
        mT = (adj_matrix_masked[b, 0].T.astype(np.uint8) * np.uint8(255))
        mT = mT.reshape(JC, 128, N)
        for iq in range(4):
            sl = slice(iq * NQ, (iq + 1) * NQ)
            xTi16 = np.ascontiguousarray(
                np.stack([xT16[:, cc * N + iq * NQ: cc * N + (iq + 1) * NQ]
                          for cc in range(2)], axis=1).reshape(128, 2 * NQ))
            mprep = np.ascontiguousarray(
                mT[:, :, sl].transpose(1, 0, 2).reshape(128, JC * NQ))
            in_maps.append(dict(xT=xT16, xTi=xTi16, waug=waug16,
                                wsb=wsb16, mprep=mprep))
    return in_maps


def _run(x, adj_matrix_masked, W, attention, reps=1):
    from concourse.bass_utils import run_bass_kernel_spmd

    key = f"nc{reps}"
    if key not in _cached:
        _cached[key] = _build(reps)
    nc = _cached[key]

    in_maps = _prep_inputs(x, adj_matrix_masked, W, attention)
    res = run_bass_kernel_spmd(nc, in_maps, core_ids=list(range(NCORES)))
    out = np.empty((B, N, D), np.float32)
    for core in range(NCORES):
        b, iq = divmod(core, 4)
        obT = res.results[core]["out"]                 # [64, NQ]
        out[b, iq * NQ:(iq + 1) * NQ] = obT.T
    return out, res


def kernel(x, adj_matrix_masked, W, attention):
    out, _ = _run(x, adj_matrix_masked, W, attention)
    return out
